# revision 3
# baseline (speedup 1.0000x reference)
"""Bilinear single-head attention (nn_Attention_73323681677530) on 8 TRN2 cores.

Sharding: pure data-parallel over batch (B=8 -> 1 batch element per core).
Weights replicated. No collectives.

Per-core dataflow (everything fp16 on the TensorEngine, fp32 PSUM accum):
  phase 0: SWDGE cast-DMAs f32->fp16 into HBM scratch (k16, q16, wk16, wq16,
           wp16); W_bil/biases loaded straight to SBUF.
  phase 1: kT tiles via DMA-transpose; kxT[e,r] = WkT.T @ kT (+bk); PE-transpose
           of kxT blocks -> kx[r,d] (both layouts needed later).
  phase 2: qT tiles -> qxT[d',q] (+bq) -> qwT[e,q] = W_bil.T @ qxT.
  phase 3: per 128-row q tile: logits PSUM = qwT.T @ kxT; row-max (DVE);
           Exp(bias=-max) on ScalarE with fused row-sum (accum_out) -> fp16 exp;
           score f32 = exp * recip(sum) -> DMA out; PE-transpose exp -> expT;
           per 512-q group: outT[d,q] = kx.T @ expT; proj PSUM = outT.T @ WpT;
           final = (proj * recip) + bp -> DMA out.
"""

import os
import sys
from contextlib import ExitStack

import numpy as np

for _p in ("/opt/trn_rl_repo", os.path.expanduser("~/.axon_site/_ro/trn_rl_repo")):
    if os.path.isdir(_p) and _p not in sys.path:
        sys.path.insert(0, _p)

import concourse.bass as bass
import concourse.tile as tile
from concourse import bacc, mybir
from concourse.masks import make_identity

F32 = mybir.dt.float32
F16 = mybir.dt.float16
P = 128


def build_graph(R=2048, D=1024, NCHUNK=512, GROUP=512):
    """Build the single-core Bacc graph (same graph runs SPMD on all 8 cores)."""
    NT = R // P        # row tiles
    ET = D // P        # embed tiles
    RC = R // NCHUNK   # row chunks for the projection matmuls
    NG = R // GROUP    # q groups for attention-value/proj
    JG = GROUP // P    # q tiles per group

    nc = bacc.Bacc("TRN2", target_bir_lowering=False, debug=False)

    k_d = nc.dram_tensor("k", [R, D], F32, kind="ExternalInput").ap()
    q_d = nc.dram_tensor("q", [R, D], F32, kind="ExternalInput").ap()
    Wk_d = nc.dram_tensor("Wk", [D, D], F32, kind="ExternalInput").ap()
    bk_d = nc.dram_tensor("bk", [D], F32, kind="ExternalInput").ap()
    Wq_d = nc.dram_tensor("Wq", [D, D], F32, kind="ExternalInput").ap()
    bq_d = nc.dram_tensor("bq", [D], F32, kind="ExternalInput").ap()
    Wbil_d = nc.dram_tensor("W_bil", [D, D], F32, kind="ExternalInput").ap()
    Wp_d = nc.dram_tensor("Wp", [D, D], F32, kind="ExternalInput").ap()
    bp_d = nc.dram_tensor("bp", [D], F32, kind="ExternalInput").ap()
    out_d = nc.dram_tensor("out", [R, D], F32, kind="ExternalOutput").ap()
    score_d = nc.dram_tensor("score", [R, R], F32, kind="ExternalOutput").ap()

    with tile.TileContext(nc) as tc, ExitStack() as ctx:
        dram = ctx.enter_context(tc.tile_pool(name="dram", bufs=1, space="DRAM"))
        consts = ctx.enter_context(tc.tile_pool(name="consts", bufs=1))
        persist = ctx.enter_context(tc.tile_pool(name="persist", bufs=1))

        # ---- phase 0: casts + constants -------------------------------------
        wk16 = dram.tile([D, D], F16, tag="wk16")
        wq16 = dram.tile([D, D], F16, tag="wq16")
        wp16 = dram.tile([D, D], F16, tag="wp16")
        k16 = dram.tile([R, D], F16, tag="k16")
        q16 = dram.tile([R, D], F16, tag="q16")
        nc.gpsimd.dma_start(out=wk16, in_=Wk_d)
        nc.gpsimd.dma_start(out=wq16, in_=Wq_d)
        nc.gpsimd.dma_start(out=wp16, in_=Wp_d)
        nc.gpsimd.dma_start(out=k16, in_=k_d)
        nc.gpsimd.dma_start(out=q16, in_=q_d)

        ident = consts.tile([P, P], F16, tag="ident")
        make_identity(nc, ident)

        # biases: b*_sb[p, t] = b[t*128 + p]
        bk_sb = consts.tile([P, ET], F32, tag="bk")
        nc.gpsimd.dma_start(out=bk_sb, in_=bk_d.rearrange("(t p) -> p t", p=P))
        bq_sb = consts.tile([P, ET], F32, tag="bq")
        nc.gpsimd.dma_start(out=bq_sb, in_=bq_d.rearrange("(t p) -> p t", p=P))
        bp_sb = consts.tile([P, D], F32, tag="bp")
        bp_bcast = bass.AP(tensor=bp_d.tensor, offset=bp_d.offset,
                           ap=[[0, P]] + bp_d.ap)
        nc.gpsimd.dma_start(out=bp_sb, in_=bp_bcast)

        # W_bil used as lhsT[d', e] directly (natural [d,e] layout), cast-load.
        wbil_sb = persist.tile([P, ET, D], F16, tag="wbil")
        nc.gpsimd.dma_start(out=wbil_sb,
                            in_=Wbil_d.rearrange("(t p) e -> p t e", p=P))

        # persistent big activations
        kxT_sb = persist.tile([P, ET, R], F16, tag="kxT")   # kxT[e, r]
        kx_sb = persist.tile([P, NT, D], F16, tag="kx")     # kx[r, d]
        qwT_sb = persist.tile([P, ET, R], F16, tag="qwT")   # qwT[e, q]
        wpT_sb = persist.tile([P, ET, D], F16, tag="wpT")   # WpT[d, e]

        # ---- phase 1: k path ------------------------------------------------
        with tc.tile_pool(name="ph1", bufs=1) as ph1, \
             tc.tile_pool(name="ph1kT", bufs=2) as ph1kT, \
             tc.tile_pool(name="mmps1", bufs=2, space="PSUM") as mmps, \
             tc.tile_pool(name="trps1", bufs=2, space="PSUM") as trps:
            wkT_sb = ph1.tile([P, ET, D], F16, tag="wkT")   # WkT[d, e]
            for dt_ in range(ET):
                nc.sync.dma_start(out=wkT_sb[:, dt_, :],
                                  in_=wk16[:, dt_ * P:(dt_ + 1) * P],
                                  transpose=True)
            for rc in range(RC):
                r0 = rc * NCHUNK
                kT_c = ph1kT.tile([P, ET, NCHUNK], F16, tag="kT")
                for dt_ in range(ET):
                    nc.sync.dma_start(out=kT_c[:, dt_, :],
                                      in_=k16[r0:r0 + NCHUNK, dt_ * P:(dt_ + 1) * P],
                                      transpose=True)
                for et in range(ET):
                    ps = mmps.tile([P, NCHUNK], F32, tag="mm")
                    for dt_ in range(ET):
                        nc.tensor.matmul(ps, wkT_sb[:, dt_, et * P:(et + 1) * P],
                                         kT_c[:, dt_, :],
                                         start=(dt_ == 0), stop=(dt_ == ET - 1))
                    # kxT chunk (+bk) -> fp16
                    nc.scalar.activation(out=kxT_sb[:, et, r0:r0 + NCHUNK], in_=ps,
                                         func=mybir.ActivationFunctionType.Identity,
                                         bias=bk_sb[:, et:et + 1])
                # kx normal layout via PE transpose of the kxT chunk
                kt0 = r0 // P
                nkt = NCHUNK // P
                for et in range(ET):
                    tp = trps.tile([P, nkt, P], F16, tag="tr")
                    for kk in range(nkt):
                        nc.tensor.transpose(
                            tp[:, kk, :],
                            kxT_sb[:, et, (kt0 + kk) * P:(kt0 + kk + 1) * P],
                            ident)
                    nc.vector.tensor_copy(
                        out=kx_sb[:, kt0:kt0 + nkt, et * P:(et + 1) * P], in_=tp)

        # ---- phase 2: q path ------------------------------------------------
        with tc.tile_pool(name="ph2", bufs=1) as ph2, \
             tc.tile_pool(name="ph2qT", bufs=2) as ph2qT, \
             tc.tile_pool(name="ph2qxT", bufs=2) as ph2qxT, \
             tc.tile_pool(name="mmps2", bufs=2, space="PSUM") as mmps:
            wqT_sb = ph2.tile([P, ET, D], F16, tag="wqT")
            for dt_ in range(ET):
                nc.sync.dma_start(out=wqT_sb[:, dt_, :],
                                  in_=wq16[:, dt_ * P:(dt_ + 1) * P],
                                  transpose=True)
            for rc in range(RC):
                r0 = rc * NCHUNK
                qT_c = ph2qT.tile([P, ET, NCHUNK], F16, tag="qT")
                for dt_ in range(ET):
                    nc.sync.dma_start(out=qT_c[:, dt_, :],
                                      in_=q16[r0:r0 + NCHUNK, dt_ * P:(dt_ + 1) * P],
                                      transpose=True)
                qxT_c = ph2qxT.tile([P, ET, NCHUNK], F16, tag="qxT")
                for d2 in range(ET):
                    ps = mmps.tile([P, NCHUNK], F32, tag="mm")
                    for dt_ in range(ET):
                        nc.tensor.matmul(ps, wqT_sb[:, dt_, d2 * P:(d2 + 1) * P],
                                         qT_c[:, dt_, :],
                                         start=(dt_ == 0), stop=(dt_ == ET - 1))
                    nc.scalar.activation(out=qxT_c[:, d2, :], in_=ps,
                                         func=mybir.ActivationFunctionType.Identity,
                                         bias=bq_sb[:, d2:d2 + 1])
                for et in range(ET):
                    ps = mmps.tile([P, NCHUNK], F32, tag="mm")
                    for d2 in range(ET):
                        nc.tensor.matmul(ps, wbil_sb[:, d2, et * P:(et + 1) * P],
                                         qxT_c[:, d2, :],
                                         start=(d2 == 0), stop=(d2 == ET - 1))
                    nc.vector.tensor_copy(out=qwT_sb[:, et, r0:r0 + NCHUNK], in_=ps)

        # WpT for phase 3
        for dt_ in range(ET):
            nc.sync.dma_start(out=wpT_sb[:, dt_, :],
                              in_=wp16[:, dt_ * P:(dt_ + 1) * P], transpose=True)

        # ---- phase 3: attention --------------------------------------------
        HB = min(1024, R)  # logits psum tile width (2 banks)
        NH = R // HB
        with tc.tile_pool(name="expT", bufs=1) as expTp, \
             tc.tile_pool(name="exp", bufs=2) as expp, \
             tc.tile_pool(name="score", bufs=2) as scorep, \
             tc.tile_pool(name="outT", bufs=2) as outTp, \
             tc.tile_pool(name="outf", bufs=2) as outfp, \
             tc.tile_pool(name="stats", bufs=2 * JG + 2) as statsp, \
             tc.tile_pool(name="lgps", bufs=2, space="PSUM") as lgps, \
             tc.tile_pool(name="trps3", bufs=2, space="PSUM") as trps, \
             tc.tile_pool(name="accps", bufs=2, space="PSUM") as accps:
            expT_sb = expTp.tile([P, NT, GROUP], F16, tag="expT")
            for g in range(NG):
                rcps = []
                exps = []
                for j in range(JG):
                    qt = g * JG + j
                    q0 = qt * P
                    lgs = []
                    nm2 = statsp.tile([P, NH], F32, tag="nm2")
                    for h in range(NH):
                        lg = lgps.tile([P, HB], F32, tag="lg")
                        for kc in range(HB // NCHUNK):
                            c0 = kc * NCHUNK
                            for et in range(ET):
                                nc.tensor.matmul(
                                    lg[:, c0:c0 + NCHUNK],
                                    qwT_sb[:, et, q0:q0 + P],
                                    kxT_sb[:, et, h * HB + c0:h * HB + c0 + NCHUNK],
                                    start=(et == 0), stop=(et == ET - 1))
                        nc.vector.reduce_max(out=nm2[:, h:h + 1], in_=lg,
                                             axis=mybir.AxisListType.X)
                        lgs.append(lg)
                    negmax = statsp.tile([P, 1], F32, tag="negmax")
                    nc.vector.reduce_max(out=negmax, in_=nm2,
                                         axis=mybir.AxisListType.X, negate=True)
                    exp_t = expp.tile([P, R], F16, tag="exp")
                    sums = statsp.tile([P, NH], F32, tag="sums")
                    for h in range(NH):
                        nc.scalar.activation(out=exp_t[:, h * HB:(h + 1) * HB],
                                             in_=lgs[h],
                                             func=mybir.ActivationFunctionType.Exp,
                                             bias=negmax,
                                             accum_out=sums[:, h:h + 1])
                    rowsum = statsp.tile([P, 1], F32, tag="rowsum")
                    nc.vector.reduce_sum(out=rowsum, in_=sums,
                                         axis=mybir.AxisListType.X)
                    rcp = statsp.tile([P, 1], F32, tag="rcp")
                    nc.vector.reciprocal(out=rcp, in_=rowsum)
                    rcps.append(rcp)
                    exps.append(exp_t)
                    # normalized score -> f32 -> HBM
                    score_t = scorep.tile([P, R], F32, tag="score")
                    nc.scalar.activation(out=score_t, in_=exp_t,
                                         func=mybir.ActivationFunctionType.Copy,
                                         scale=rcp)
                    nc.sync.dma_start(out=score_d[q0:q0 + P, :], in_=score_t)
                # transpose exp tiles of the group -> expT[k, qg]
                for j in range(JG):
                    exp_t = exps[j]
                    for ktg in range(NT // 4):
                        tp = trps.tile([P, 4, P], F16, tag="tr")
                        for kk in range(4):
                            kt = ktg * 4 + kk
                            nc.tensor.transpose(tp[:, kk, :],
                                                exp_t[:, kt * P:(kt + 1) * P],
                                                ident)
                        nc.vector.tensor_copy(
                            out=expT_sb[:, ktg * 4:(ktg + 1) * 4, j * P:(j + 1) * P],
                            in_=tp)
                # attention-value: outT[d, qg] (unnormalized)
                outT_c = outTp.tile([P, ET, GROUP], F16, tag="outT")
                for dt_ in range(ET):
                    ps = accps.tile([P, GROUP], F32, tag="acc")
                    for kt in range(NT):
                        nc.tensor.matmul(ps, kx_sb[:, kt, dt_ * P:(dt_ + 1) * P],
                                         expT_sb[:, kt, :],
                                         start=(kt == 0), stop=(kt == NT - 1))
                    nc.vector.tensor_copy(out=outT_c[:, dt_, :], in_=ps)
                # projection + fused (x recip + bp) epilogue
                for j in range(JG):
                    qt = g * JG + j
                    q0 = qt * P
                    outf = outfp.tile([P, D], F32, tag="outf")
                    for ec in range(D // NCHUNK):
                        e0 = ec * NCHUNK
                        ps = accps.tile([P, NCHUNK], F32, tag="acc")
                        for dt_ in range(ET):
                            nc.tensor.matmul(ps, outT_c[:, dt_, j * P:(j + 1) * P],
                                             wpT_sb[:, dt_, e0:e0 + NCHUNK],
                                             start=(dt_ == 0), stop=(dt_ == ET - 1))
                        nc.vector.scalar_tensor_tensor(
                            out=outf[:, e0:e0 + NCHUNK], in0=ps, scalar=rcps[j],
                            in1=bp_sb[:, e0:e0 + NCHUNK],
                            op0=mybir.AluOpType.mult, op1=mybir.AluOpType.add)
                    nc.sync.dma_start(out=out_d[q0:q0 + P, :], in_=outf)

    nc.compile()
    return nc


_CACHE = {}


def _get_graph():
    if "nc" not in _CACHE:
        _CACHE["nc"] = build_graph()
    return _CACHE["nc"]


def kernel(k, q, Wk, bk, Wq, bq, W_bil, Wp, bp):
    from concourse.bass_utils import run_bass_kernel_spmd

    nc = _get_graph()
    k = np.asarray(k, dtype=np.float32)
    q = np.asarray(q, dtype=np.float32)
    shared = {
        "Wk": np.ascontiguousarray(Wk, dtype=np.float32),
        "bk": np.ascontiguousarray(bk, dtype=np.float32),
        "Wq": np.ascontiguousarray(Wq, dtype=np.float32),
        "bq": np.ascontiguousarray(bq, dtype=np.float32),
        "W_bil": np.ascontiguousarray(W_bil, dtype=np.float32),
        "Wp": np.ascontiguousarray(Wp, dtype=np.float32),
        "bp": np.ascontiguousarray(bp, dtype=np.float32),
    }
    n_cores = 8
    in_maps = [
        {"k": np.ascontiguousarray(k[i]), "q": np.ascontiguousarray(q[i]), **shared}
        for i in range(n_cores)
    ]
    trace = bool(int(os.environ.get("BASS_KERNEL_TRACE", "0")))
    res = run_bass_kernel_spmd(nc, in_maps, core_ids=list(range(n_cores)),
                               trace=trace)
    _CACHE["last_result"] = res
    out = np.stack([res.results[i]["out"] for i in range(n_cores)])
    score = np.stack([res.results[i]["score"] for i in range(n_cores)])
    return out, score
